# revision 43
# baseline (speedup 1.0000x reference)
"""EvolveGCN Trainium2 kernel (8-core SPMD), aggregate-first design.

Math: out_l = rrelu(segment_sum(w_e * x_l[src_e]) @ Q_l) -- the GCN weight
multiply commutes with the (linear) edge aggregation, so we aggregate raw
feature rows first and apply Q to the [N, F] aggregate afterwards.

Strategy:
  - Nodes/edges sharded by destination across 8 cores (6250 nodes each).
  - GRU weight evolution (128x128 mats) computed on the host.
  - Layer 1 payload rows are pre-gathered on the HOST into exact
    (lane, chunk) order (the gather indices are pure input data), so the
    device just streams them with large contiguous DMAs -- zero SWDGE
    descriptor generation and no small-descriptor HBM penalty for layer 1.
  - Layer 2 needs the layer-1 output of all cores: one fp8(e3m4) AllGather
    per timestep, then true dma_gather (device-dependent data).  The fp8
    table is addressed as 256 B row PAIRS (even-parity node at row 2k, odd
    at 2k+1; the packing forces pos parity == node parity), so pair indices
    fit int16 with no table split and each chunk's class (= source parity)
    statically picks which feature half of the gathered pair feeds the PE.
  - Aggregation uses PE matmuls: gathered 128-edge chunks (lhsT) times
    host-built fp8 one-hot selection matrices S (rhs, edge-weight folded
    in) accumulate into per-slot PSUM columns (accumulation groups kept
    contiguous per slot -- interleaved open groups corrupt PSUM).
  - The @Q product then maps the [F, pos] aggregate back to node-major
    [pos, F] PSUM tiles, which store as contiguous rows (bf16 output,
    upcast on host).
  - Host bin-packs destination nodes into fixed-capacity slots (64 node
    positions, 32 even + 32 odd, 4+4 chunks of 128 edges for the two
    source-parity classes) so the device program is fully static; padding
    edges have w=0 and spread-out gather indices.
  - Emission interleaves timesteps (L1(t) ... L2(t-1) p0, AG(t),
    L2(t-1) p1-6) so the gather queue never head-of-line-stalls on the
    collective trigger and the remaining passes hide its latency.
  - dynamic_dma_scratch_size=24576 deepens the SWDGE descriptor rings so
    the gather stream rides out drain-pacing bursts.
"""

import sys

for _p in ("/opt/trn_rl_repo", "/opt/pypackages"):
    if _p not in sys.path:
        sys.path.append(_p)

from dataclasses import dataclass

import numpy as np
import ml_dtypes

BF16 = ml_dtypes.bfloat16
NEG_SLOPE = (1.0 / 8.0 + 1.0 / 3.0) / 2.0


@dataclass(frozen=True)
class Cfg:
    T: int = 6
    N: int = 50000
    F: int = 128
    L: int = 2
    NCORES: int = 8
    POS: int = 64         # node positions per slot
    # chunks (of 128 edges) per slot per table half, by pass: the last pass
    # is half-capacity, trimming total edge slots 114688 -> 106496 (~7%)
    PCAPS: tuple = (4, 4, 4, 4, 4, 3, 3)
    SLOTS: int = 16       # slots per psum pass
    NPASS: int = 7

    @property
    def NPC(self):  # nodes per core (dst sharding)
        return self.N // self.NCORES

    @property
    def NSLOT(self):
        return self.SLOTS * self.NPASS

    @property
    def POS_TOT(self):  # padded positions per core
        return self.NSLOT * self.POS

    @property
    def PASS_W(self):  # psum width per pass
        return self.SLOTS * self.POS

    @property
    def CHT(self):  # max chunks per pass (A block + B block)
        return 2 * max(self.PCAPS) * self.SLOTS

    @property
    def CHTOT(self):  # total chunks over all passes
        return 2 * sum(self.PCAPS) * self.SLOTS

    @property
    def IDXTOT(self):  # total idx columns over all passes (A+B)
        return self.CHTOT * 8

    def chh(self, p):  # chunks per half in pass p
        return self.PCAPS[p] * self.SLOTS

    def idxoff(self, p):  # idx column offset of pass p
        return 2 * 8 * self.SLOTS * sum(self.PCAPS[:p])

    def soff(self, p):  # S element offset of pass p
        return 2 * self.SLOTS * self.POS * sum(self.PCAPS[:p])

    @property
    def HALF1(self):  # layer-1 gather table half size
        return self.N // 2

    @property
    def HALF2(self):  # layer-2 table half size (= cores 0..NC/2-1)
        return self.NCORES * self.POS_TOT // 2


CFG = Cfg()


# ----------------------------------------------------------------- host math

def host_gru(gate_W, gate_U, gate_b, W0, T):
    """Evolve the GCN weight through the GRU on the host.

    Returns Q[l, t] float32 [L, T, F, F]."""
    L = gate_W.shape[0]
    F = W0.shape[-1]
    out = np.zeros((L, T, F, F), dtype=np.float32)

    def sigmoid(v):
        return 1.0 / (1.0 + np.exp(-v))

    for l in range(L):
        Q = W0[l].astype(np.float32)
        gW, gU, gb = (np.asarray(a[l], dtype=np.float32) for a in (gate_W, gate_U, gate_b))
        for t in range(T):
            z = sigmoid(gW[0] @ Q + gU[0] @ Q + gb[0])
            r = sigmoid(gW[1] @ Q + gU[1] @ Q + gb[1])
            h = np.tanh(gW[2] @ Q + gU[2] @ (r * Q) + gb[2])
            Q = (1.0 - z) * Q + z * h
            out[l, t] = Q
    return out


def pack_core_t(dst_local, col, w, cfg: Cfg):
    """Bin-pack one core's edges at one timestep into the static slot layout.

    Returns (pos, perms) where
      pos:   [NPC] position assigned to each local node
      perms: per (pass, half) -> int array of edge ids (-1 for padding) laid
             out as [CH*128] in chunk order (slot-major).
    """
    # Edge class (A/B chunk block) = parity of the SOURCE node id.  The
    # layer-2 gather table holds fp8 row PAIRS (even-parity node at row 2k,
    # odd at 2k+1), so a 256 B gather entry serves either class and the
    # chunk's class picks which feature half of the pair to feed the PE --
    # a compile-time constant, identical on every core.
    half = (col % 2).astype(np.int8)

    # per-node degree split by class
    degA = np.bincount(dst_local[half == 0], minlength=cfg.NPC)
    degB = np.bincount(dst_local[half == 1], minlength=cfg.NPC)

    caps = np.repeat(np.asarray(cfg.PCAPS, dtype=np.int64), cfg.SLOTS) * 128
    remA = caps.copy()
    remB = caps.copy()
    # positions of each parity per slot (node parity must equal pos parity)
    remNe = np.full(cfg.NSLOT, cfg.POS // 2, dtype=np.int64)
    remNo = np.full(cfg.NSLOT, cfg.POS // 2, dtype=np.int64)

    order = np.argsort(-(degA + degB), kind="stable")
    slot_of = np.empty(cfg.NPC, dtype=np.int64)
    pos_in_slot = np.empty(cfg.NPC, dtype=np.int64)
    for n in order:
        da, db = degA[n], degB[n]
        remN = remNe if n % 2 == 0 else remNo
        ok = np.flatnonzero((remA >= da) & (remB >= db) & (remN > 0))
        if ok.size == 0:
            raise RuntimeError("bin packing failed; increase capacity")
        s = ok[0]
        slot_of[n] = s
        if n % 2 == 0:
            pos_in_slot[n] = 2 * (cfg.POS // 2 - remNe[s])
            remNe[s] -= 1
        else:
            pos_in_slot[n] = 2 * (cfg.POS // 2 - remNo[s]) + 1
            remNo[s] -= 1
        remA[s] -= da
        remB[s] -= db
    pos = slot_of * cfg.POS + pos_in_slot

    # assign edges to chunk lanes: order edges by (slot, half)
    eslot = slot_of[dst_local]
    key = eslot * 2 + half
    eorder = np.argsort(key, kind="stable")
    counts = np.bincount(key, minlength=cfg.NSLOT * 2)
    starts = np.concatenate(([0], np.cumsum(counts)))

    perms = []
    for p in range(cfg.NPASS):
        for h in (0, 1):
            ch = cfg.PCAPS[p]
            blk = np.full(cfg.SLOTS * ch * 128, -1, dtype=np.int64)
            for si in range(cfg.SLOTS):
                s = p * cfg.SLOTS + si
                k = s * 2 + h
                seg = eorder[starts[k]:starts[k + 1]]
                assert seg.size <= ch * 128
                blk[si * ch * 128: si * ch * 128 + seg.size] = seg
            perms.append(blk)
    return pos, perms


def build_edge_arrays(dst_local, col, w, pos_global_fn, pos, perms, xf8_t, cfg: Cfg):
    """Build per-(core,t) device arrays from a packing.

    Returns dict with
      xg:   [128, CHTOT, F] float8_e3m4 layer-1 edge payloads, pre-gathered
            on the host into exact (lane, chunk) order -- the device streams
            these with plain contiguous DMA (zero gather descriptors for
            layer 1)
      idx2: list of [128, nch*8] int16 (order pass,half) layer-2 gather idx
      s8:   one-hot selection matrices [128, CHTOT, POS] float8_e3m4 holding
            8*w at each edge's destination position (the 8x keeps small
            weights out of the fp8 denormal range; the host folds 1/8 into
            Q).
    """
    col2 = pos_global_fn(col)
    F8 = ml_dtypes.float8_e3m4
    s8 = np.zeros((128, cfg.CHTOT, cfg.POS), dtype=F8)
    xg = np.zeros((128, cfg.CHTOT, cfg.F), dtype=F8)
    idx2_out = []

    ci = 0  # global chunk index in [0, CHTOT)
    for p in range(cfg.NPASS):
        for bi, h in enumerate((0, 1)):
            ch = cfg.PCAPS[p]
            blk = perms[p * 2 + bi]
            nch = cfg.SLOTS * ch
            e = blk.reshape(nch, 128)
            valid = e >= 0
            esafe = np.where(valid, e, 0)

            # layer-1 payload rows, pre-gathered host-side (pads stay zero)
            rows = xf8_t[col[esafe]]  # [nch, 128, F]
            rows[~valid] = 0
            xg[:, ci:ci + nch, :] = rows.transpose(1, 0, 2)

            # layer-2 gather indices: fp8 row-pair entries; pads get
            # spread-out indices (identical pad indices hot-spot one bank)
            c_l2 = col2[esafe]
            assert np.all((c_l2 % 2)[valid] == h), "pos parity != src parity"
            spread = (np.arange(e.size, dtype=np.int64).reshape(e.shape) * 2654435761)
            i2 = np.where(valid, c_l2 // 2, spread % cfg.HALF2).astype(np.int64)
            assert i2.min() >= 0 and i2.max() < cfg.HALF2

            def wrap(ix):
                # ix [nch, 128] in chunk order -> flat order i -> tile[i%16, i//16]
                flat = ix.reshape(-1)
                tile = flat.reshape(-1, 16).T.astype(np.int16)  # [16, nch*8]
                return np.tile(tile, (8, 1))  # replicate to 128 partitions

            idx2_out.append(wrap(i2))

            # S[lane, ci+c, dstpos] = 8*w (padding lanes keep w=0)
            dl = np.where(valid, pos[dst_local[esafe]] % cfg.POS, 0)
            wv = np.where(valid, 8.0 * w[esafe], 0.0).astype(F8)
            lanes = np.broadcast_to(np.arange(128)[None, :], e.shape)
            cs = np.broadcast_to(np.arange(nch)[:, None], e.shape)
            s8[lanes.ravel(), (ci + cs).ravel(), dl.ravel()] = wv.ravel()
            ci += nch

    return {
        "xg": xg,
        "idx2": idx2_out,
        "s8": s8.reshape(128, -1),
    }


def host_preprocess(x, edge_index, edge_weight, gate_W, gate_U, gate_b, W0, cfg: Cfg):
    """Build all per-core device inputs. Returns (in_maps, meta)."""
    T, N, F = x.shape
    q = host_gru(gate_W, gate_U, gate_b, W0, T)  # [L,T,F,F] f32

    dst = np.asarray(edge_index[:, 0], dtype=np.int64)
    col = np.asarray(edge_index[:, 1], dtype=np.int64)
    w = np.asarray(edge_weight, dtype=np.float32)

    owner = dst // cfg.NPC

    # phase 1: pack every (core, t); collect pos maps
    pos_all = np.zeros((T, N), dtype=np.int64)
    packs = {}
    for t in range(T):
        for c in range(cfg.NCORES):
            m = owner[t] == c
            dl = dst[t][m] - c * cfg.NPC
            pos, perms = pack_core_t(dl, col[t][m], w[t][m], cfg)
            packs[(t, c)] = (dl, col[t][m], w[t][m], pos, perms)
            pos_all[t, c * cfg.NPC: (c + 1) * cfg.NPC] = pos

    # phase 2: per-core arrays.  The 8x edge-weight scaling inside S is
    # compensated by an 1/8 on every Q.
    qbf = (q / 8.0).astype(BF16)  # [L,T,F,F]
    xf8 = np.ascontiguousarray(x.astype(ml_dtypes.float8_e3m4))  # payload src

    in_maps = []
    for c in range(cfg.NCORES):
        xg_l, idx2_l, s8_l = [], [], []
        for t in range(T):
            dl, ct, wt, pos, perms = packs[(t, c)]

            def pos_global(carr, t=t):
                own = carr // cfg.NPC
                return own * cfg.POS_TOT + pos_all[t, carr]

            arrs = build_edge_arrays(dl, ct, wt, pos_global, pos, perms,
                                     xf8[t], cfg)
            xg_l.append(arrs["xg"])
            idx2_l.append(arrs["idx2"])
            s8_l.append(arrs["s8"])

        # idx tensor layout: [T, 128, IDXTOT] flat, per pass (A block then B)
        idx2 = np.zeros((T, 128, cfg.IDXTOT), dtype=np.int16)
        for t in range(T):
            for p in range(cfg.NPASS):
                o = cfg.idxoff(p)
                ch8 = cfg.chh(p) * 8
                idx2[t, :, o:o + ch8] = idx2_l[t][p * 2]
                idx2[t, :, o + ch8:o + 2 * ch8] = idx2_l[t][p * 2 + 1]

        in_maps.append({
            "xg": np.stack(xg_l),    # [T, 128, CHTOT, F] float8_e3m4
            "qmat": qbf,
            "idx2": idx2,
            "sfp8": np.stack(s8_l),  # [T, 128, CHTOT*POS] float8_e3m4
        })

    meta = {"pos_all": pos_all}
    return in_maps, meta


def host_assemble(results, pos_all, cfg: Cfg):
    """results: per-core dicts with 'out' [T, POS_TOT, F] f32 -> [T,N,F] f32."""
    T, N = pos_all.shape
    out = np.zeros((T, N, cfg.F), dtype=np.float32)
    for c, r in enumerate(results):
        dev = r["out"]  # [T, POS_TOT, F]
        for t in range(T):
            p = pos_all[t, c * cfg.NPC:(c + 1) * cfg.NPC]
            out[t, c * cfg.NPC:(c + 1) * cfg.NPC, :] = dev[t][p, :]
    return out


# ------------------------------------------------------------- bass program

def build_bass(cfg: Cfg):
    import concourse.mybir as mybir
    import concourse.tile as tile
    from concourse.bacc import Bacc

    fp32 = mybir.dt.float32
    bf16 = mybir.dt.bfloat16
    i16 = mybir.dt.int16
    f8e3 = mybir.dt.float8e3
    AF = mybir.ActivationFunctionType

    nc = Bacc(num_swdge_queues=4, dynamic_dma_scratch_size=24576)
    T, F = cfg.T, cfg.F

    xg = nc.declare_dram_parameter("xg", [T, 128, cfg.CHTOT, F], f8e3, isOutput=False)
    qmat = nc.declare_dram_parameter("qmat", [cfg.L, T, F, F], bf16, isOutput=False)
    idx2 = nc.declare_dram_parameter("idx2", [T, 128, cfg.IDXTOT], i16, isOutput=False)
    sfp8_in = nc.declare_dram_parameter(
        "sfp8", [T, 128, cfg.CHTOT * cfg.POS], f8e3, isOutput=False)
    out_d = nc.declare_dram_parameter("out", [T, cfg.POS_TOT, F], bf16, isOutput=True)

    # per-timestep scratch: layer-1 output slice + its AllGather
    t2own = nc.dram_tensor("t2own", [T, cfg.POS_TOT, F], f8e3)
    kw = {"addr_space": "Shared"} if cfg.NCORES > 4 else {}
    t2full = nc.dram_tensor("t2full", [T, cfg.NCORES * cfg.POS_TOT, F], f8e3, **kw)
    groups = [list(range(cfg.NCORES))]

    with tile.TileContext(nc) as tc:
        with (
            tc.tile_pool(name="const", bufs=1) as constp,
            tc.tile_pool(name="meta", bufs=2) as metap,
            tc.tile_pool(name="msgf8", bufs=2) as msgf8p,
            tc.tile_pool(name="msgbf", bufs=4) as msgbfp,
            tc.tile_pool(name="sb", bufs=2) as sp,
            tc.tile_pool(name="stg", bufs=2) as stgp,
            tc.tile_pool(name="apsum", bufs=1, space="PSUM") as psp,
            tc.tile_pool(name="qpsum", bufs=4, space="PSUM") as hps,
        ):
            q_t = constp.tile([128, cfg.L * T * F], bf16)
            for l in range(cfg.L):
                for t in range(T):
                    o = (l * T + t) * F
                    nc.sync.dma_start(out=q_t[:, o:o + F], in_=qmat[l, t, :, :])

            qctr = [0]

            def qsl(l, t):
                o = (l * T + t) * F
                return q_t[:, o:o + F]

            def emit_pass(t, l, p):
                halfsz = cfg.HALF2
                chh = cfg.chh(p)  # chunks per half this pass
                cap = cfg.PCAPS[p]
                io = cfg.idxoff(p)
                so = cfg.soff(p)
                co = so // cfg.POS  # chunk offset of pass p

                if l == 0:
                    # layer 1: host pre-gathered payload, plain streaming
                    msg = msgf8p.tile([128, cfg.CHT, F], f8e3, tag="msg8")
                    nc.sync.dma_start(
                        out=msg[:, :2 * chh, :],
                        in_=xg[t, :, co:co + 2 * chh, :])
                else:
                    # layer 2: each 256 B entry is an (even,odd)-node fp8
                    # row pair; the chunk's class picks the feature half
                    msg = msgbfp.tile([128, cfg.CHT, 2 * F], f8e3, tag="msgb")
                    idxt = metap.tile([128, cfg.CHT * 8], i16, tag="idx")
                    nc.sync.dma_start(
                        out=idxt[:, :2 * chh * 8],
                        in_=idx2[t, :, io:io + 2 * chh * 8])
                    pairs = t2full[t, :, :].rearrange(
                        "(r two) f -> r (two f)", two=2)
                    for h in (0, 1):
                        nc.gpsimd.dma_gather(
                            msg[:, h * chh:(h + 1) * chh, :],
                            pairs,
                            idxt[:, h * chh * 8:(h + 1) * chh * 8],
                            num_idxs=chh * 128,
                            num_idxs_reg=chh * 128,
                            elem_size=2 * F,
                            single_packet=False,
                            queue_num=qctr[0] % 4,
                        )
                        qctr[0] += 1

                # one-hot S with 8*w folded in, host-built fp8 (e3m4)
                S = sp.tile([128, cfg.CHT, cfg.POS], f8e3, tag="S")
                nc.sync.dma_start(
                    out=S[:, :2 * chh, :],
                    in_=sfp8_in[t, :, so:so + 2 * chh * cfg.POS].rearrange(
                        "p (c j) -> p c j", j=cfg.POS),
                )

                # aggregate per table half into separate PSUM tiles so the
                # A-half matmul burst overlaps the B gather's tail; the Q
                # matmul below sums the halves via PE accumulation.
                bsbh = []
                for h in (0, 1):
                    agg = psp.tile([128, cfg.PASS_W], fp32, tag=f"agg{h}")
                    for si in range(cfg.SLOTS):
                        for k in range(cap):
                            ci = h * chh + si * cap + k
                            lhsT = (msg[:, ci, :] if l == 0 else
                                    msg[:, ci, h * F:(h + 1) * F])
                            nc.tensor.matmul(
                                out=agg[:, si * cfg.POS:(si + 1) * cfg.POS],
                                lhsT=lhsT,
                                rhs=S[:, ci, :],
                                start=(k == 0),
                                stop=(k == cap - 1),
                            )
                    bsb = sp.tile([128, cfg.PASS_W], bf16, tag=f"bsb{h}")
                    nc.scalar.activation(out=bsb[:], in_=agg[:], func=AF.Copy)
                    bsbh.append(bsb)
                emit_tail(t, l, p, bsbh)

            def emit_tail(t, l, p, bsbh):
                # node-major (B @ Q) in 128-row blocks, rrelu, store; the two
                # aggregation halves sum via a 2-matmul accumulation group
                for j0 in range(0, cfg.PASS_W // 128, 4):
                    qp = hps.tile([128, 4 * F], fp32, tag="qp")
                    for jj in range(4):
                        j = j0 + jj
                        for h, bsb in enumerate(bsbh):
                            nc.tensor.matmul(
                                out=qp[:, jj * F:(jj + 1) * F],
                                lhsT=bsb[:, j * 128:(j + 1) * 128],
                                rhs=qsl(l, t),
                                start=(h == 0), stop=(h == len(bsbh) - 1),
                            )
                    lk = stgp.tile([128, 4 * F], fp32, tag="lk")
                    nc.scalar.activation(out=lk[:], in_=qp[:], func=AF.Copy, scale=NEG_SLOPE)
                    r0 = p * cfg.PASS_W + j0 * 128
                    # store via the Scalar engine so the Sync queue stays
                    # pure-loads (no head-of-line blocking of S/idx loads
                    # behind compute-dependent stores)
                    if l == 0:
                        st = stgp.tile([128, 4 * F], f8e3, tag="st1")
                        dest = t2own[t, r0:r0 + 512, :]
                    else:
                        st = stgp.tile([128, 4 * F], bf16, tag="st2")
                        dest = out_d[t, r0:r0 + 512, :]
                    nc.vector.tensor_tensor(
                        out=st[:], in0=qp[:], in1=lk[:], op=mybir.AluOpType.max,
                    )
                    nc.scalar.dma_start(
                        out=dest.rearrange("(j p) f -> p j f", p=128),
                        in_=st[:].rearrange("p (j f) -> p j f", j=4),
                    )

            # Emission order: L1(t) streams while L2(t-1) gathers; the
            # AllGather(t) trigger (a GpSimd-queue instruction that waits on
            # L1(t)'s stores) is sandwiched between early L2(t-1) passes so
            # the gather queue never head-of-line-stalls on it, while the
            # remaining L2(t-1) passes cover the collective's latency before
            # L2(t) needs t2full[t].
            def emit_ag(t):
                nc.gpsimd.collective_compute(
                    "AllGather", mybir.AluOpType.bypass,
                    replica_groups=groups,
                    ins=[t2own[t, :, :]], outs=[t2full[t, :, :]],
                )

            for t in range(T):
                for p in range(cfg.NPASS):
                    emit_pass(t, 0, p)
                if t == 0:
                    emit_ag(0)
                else:
                    emit_pass(t - 1, 1, 0)
                    emit_ag(t)
                    for p in range(1, cfg.NPASS):
                        emit_pass(t - 1, 1, p)
            for p in range(cfg.NPASS):
                emit_pass(T - 1, 1, p)
    nc.finalize()
    return nc


# ------------------------------------------------------------------- driver

TRACE = False
LAST_RESULT = None


def kernel(x, edge_index, edge_weight, gate_W, gate_U, gate_b, W0):
    global LAST_RESULT
    from concourse.bass_utils import run_bass_kernel_spmd

    cfg = CFG
    x = np.asarray(x)
    in_maps, meta = host_preprocess(
        x, np.asarray(edge_index), np.asarray(edge_weight),
        np.asarray(gate_W), np.asarray(gate_U), np.asarray(gate_b),
        np.asarray(W0), cfg,
    )
    nc = build_bass(cfg)
    res = run_bass_kernel_spmd(nc, in_maps, list(range(cfg.NCORES)), trace=TRACE)
    LAST_RESULT = res
    return host_assemble(res.results, meta["pos_all"], cfg).astype(np.float32)



# revision 44
# speedup vs baseline: 1.1981x; 1.1981x over previous
"""EvolveGCN Trainium2 kernel (8-core SPMD), aggregate-first design.

Math: out_l = rrelu(segment_sum(w_e * x_l[src_e]) @ Q_l) -- the GCN weight
multiply commutes with the (linear) edge aggregation, so we aggregate raw
feature rows first and apply Q to the [N, F] aggregate afterwards.

Strategy:
  - Nodes/edges sharded by destination across 8 cores (6250 nodes each).
  - GRU weight evolution (128x128 mats) computed on the host.
  - Layer 1 payload rows are pre-gathered on the HOST into exact
    (lane, chunk) order (the gather indices are pure input data), so the
    device just streams them with large contiguous DMAs -- zero SWDGE
    descriptor generation and no small-descriptor HBM penalty for layer 1.
  - Layer 2 needs the layer-1 output of all cores: one bf16 AllGather per
    timestep, then true dma_gather (device-dependent data).
  - Aggregation uses PE matmuls: gathered 128-edge chunks (lhsT) times
    one-hot selection matrices S (rhs, edge-weight folded in) accumulate
    into per-slot PSUM columns.  S is built on the Vector engine from
    per-edge (slot-position, weight) metadata via iota compare.
  - The @Q product then maps the [F, pos] aggregate back to node-major
    [pos, F] PSUM tiles, which store as contiguous rows.
  - Host bin-packs destination nodes into fixed-capacity slots (64 node
    positions, 4+4 chunks of 128 edges for the two gather-table halves --
    int16 gather indices force a half split) so the device program is
    fully static; padding edges have w=0 and spread-out gather indices.
  - Emission interleaves timesteps (L1(t) ... AG(t) ... L2(t-1)) so the
    gather queues stay saturated while collectives run.
"""

import sys

for _p in ("/opt/trn_rl_repo", "/opt/pypackages"):
    if _p not in sys.path:
        sys.path.append(_p)

from dataclasses import dataclass

import numpy as np
import ml_dtypes

BF16 = ml_dtypes.bfloat16
NEG_SLOPE = (1.0 / 8.0 + 1.0 / 3.0) / 2.0


@dataclass(frozen=True)
class Cfg:
    T: int = 6
    N: int = 50000
    F: int = 128
    L: int = 2
    NCORES: int = 8
    POS: int = 64         # node positions per slot
    # chunks (of 128 edges) per slot per table half, by pass: the last pass
    # is half-capacity, trimming total edge slots 114688 -> 106496 (~7%)
    PCAPS: tuple = (4, 4, 4, 4, 4, 3, 3)
    SLOTS: int = 16       # slots per psum pass
    NPASS: int = 7

    @property
    def NPC(self):  # nodes per core (dst sharding)
        return self.N // self.NCORES

    @property
    def NSLOT(self):
        return self.SLOTS * self.NPASS

    @property
    def POS_TOT(self):  # padded positions per core
        return self.NSLOT * self.POS

    @property
    def PASS_W(self):  # psum width per pass
        return self.SLOTS * self.POS

    @property
    def CHT(self):  # max chunks per pass (A block + B block)
        return 2 * max(self.PCAPS) * self.SLOTS

    @property
    def CHTOT(self):  # total chunks over all passes
        return 2 * sum(self.PCAPS) * self.SLOTS

    @property
    def IDXTOT(self):  # total idx columns over all passes (A+B)
        return self.CHTOT * 8

    def chh(self, p):  # chunks per half in pass p
        return self.PCAPS[p] * self.SLOTS

    def idxoff(self, p):  # idx column offset of pass p
        return 2 * 8 * self.SLOTS * sum(self.PCAPS[:p])

    def soff(self, p):  # S element offset of pass p
        return 2 * self.SLOTS * self.POS * sum(self.PCAPS[:p])

    @property
    def HALF1(self):  # layer-1 gather table half size
        return self.N // 2

    @property
    def HALF2(self):  # layer-2 table half size (= cores 0..NC/2-1)
        return self.NCORES * self.POS_TOT // 2


CFG = Cfg()


# ----------------------------------------------------------------- host math

def host_gru(gate_W, gate_U, gate_b, W0, T):
    """Evolve the GCN weight through the GRU on the host.

    Returns Q[l, t] float32 [L, T, F, F]."""
    L = gate_W.shape[0]
    F = W0.shape[-1]
    out = np.zeros((L, T, F, F), dtype=np.float32)

    def sigmoid(v):
        return 1.0 / (1.0 + np.exp(-v))

    for l in range(L):
        Q = W0[l].astype(np.float32)
        gW, gU, gb = (np.asarray(a[l], dtype=np.float32) for a in (gate_W, gate_U, gate_b))
        for t in range(T):
            z = sigmoid(gW[0] @ Q + gU[0] @ Q + gb[0])
            r = sigmoid(gW[1] @ Q + gU[1] @ Q + gb[1])
            h = np.tanh(gW[2] @ Q + gU[2] @ (r * Q) + gb[2])
            Q = (1.0 - z) * Q + z * h
            out[l, t] = Q
    return out


def pack_core_t(dst_local, col, w, cfg: Cfg):
    """Bin-pack one core's edges at one timestep into the static slot layout.

    Returns (pos, perms) where
      pos:   [NPC] position assigned to each local node
      perms: per (pass, half) -> int array of edge ids (-1 for padding) laid
             out as [CH*128] in chunk order (slot-major).
    """
    # Edge class (A/B chunk block) = parity of the SOURCE node id.  The
    # layer-2 gather table holds fp8 row PAIRS (even-parity node at row 2k,
    # odd at 2k+1), so a 256 B gather entry serves either class and the
    # chunk's class picks which feature half of the pair to feed the PE --
    # a compile-time constant, identical on every core.
    half = (col % 2).astype(np.int8)

    # per-node degree split by class
    degA = np.bincount(dst_local[half == 0], minlength=cfg.NPC)
    degB = np.bincount(dst_local[half == 1], minlength=cfg.NPC)

    caps = np.repeat(np.asarray(cfg.PCAPS, dtype=np.int64), cfg.SLOTS) * 128
    remA = caps.copy()
    remB = caps.copy()
    # positions of each parity per slot (node parity must equal pos parity)
    remNe = np.full(cfg.NSLOT, cfg.POS // 2, dtype=np.int64)
    remNo = np.full(cfg.NSLOT, cfg.POS // 2, dtype=np.int64)

    order = np.argsort(-(degA + degB), kind="stable")
    slot_of = np.empty(cfg.NPC, dtype=np.int64)
    pos_in_slot = np.empty(cfg.NPC, dtype=np.int64)
    for n in order:
        da, db = degA[n], degB[n]
        remN = remNe if n % 2 == 0 else remNo
        ok = np.flatnonzero((remA >= da) & (remB >= db) & (remN > 0))
        if ok.size == 0:
            raise RuntimeError("bin packing failed; increase capacity")
        s = ok[0]
        slot_of[n] = s
        if n % 2 == 0:
            pos_in_slot[n] = 2 * (cfg.POS // 2 - remNe[s])
            remNe[s] -= 1
        else:
            pos_in_slot[n] = 2 * (cfg.POS // 2 - remNo[s]) + 1
            remNo[s] -= 1
        remA[s] -= da
        remB[s] -= db
    pos = slot_of * cfg.POS + pos_in_slot

    # assign edges to chunk lanes: order edges by (slot, half)
    eslot = slot_of[dst_local]
    key = eslot * 2 + half
    eorder = np.argsort(key, kind="stable")
    counts = np.bincount(key, minlength=cfg.NSLOT * 2)
    starts = np.concatenate(([0], np.cumsum(counts)))

    perms = []
    for p in range(cfg.NPASS):
        for h in (0, 1):
            ch = cfg.PCAPS[p]
            blk = np.full(cfg.SLOTS * ch * 128, -1, dtype=np.int64)
            for si in range(cfg.SLOTS):
                s = p * cfg.SLOTS + si
                k = s * 2 + h
                seg = eorder[starts[k]:starts[k + 1]]
                assert seg.size <= ch * 128
                blk[si * ch * 128: si * ch * 128 + seg.size] = seg
            perms.append(blk)
    return pos, perms


def build_edge_arrays(dst_local, col, w, pos_global_fn, pos, perms, xf8_t, cfg: Cfg):
    """Build per-(core,t) device arrays from a packing.

    Returns dict with
      xg:   [128, CHTOT, F] float8_e3m4 layer-1 edge payloads, pre-gathered
            on the host into exact (lane, chunk) order -- the device streams
            these with plain contiguous DMA (zero gather descriptors for
            layer 1)
      idx2: list of [128, nch*8] int16 (order pass,half) layer-2 gather idx
      s8:   one-hot selection matrices [128, CHTOT, POS] float8_e3m4 holding
            8*w at each edge's destination position (the 8x keeps small
            weights out of the fp8 denormal range; the host folds 1/8 into
            Q).
    """
    col2 = pos_global_fn(col)
    F8 = ml_dtypes.float8_e3m4
    s8 = np.zeros((128, cfg.CHTOT, cfg.POS), dtype=F8)
    xg = np.zeros((128, cfg.CHTOT, cfg.F), dtype=F8)
    idx2_out = []

    ci = 0  # global chunk index in [0, CHTOT)
    for p in range(cfg.NPASS):
        for bi, h in enumerate((0, 1)):
            ch = cfg.PCAPS[p]
            blk = perms[p * 2 + bi]
            nch = cfg.SLOTS * ch
            e = blk.reshape(nch, 128)
            valid = e >= 0
            esafe = np.where(valid, e, 0)

            # layer-1 payload rows, pre-gathered host-side (pads stay zero)
            rows = xf8_t[col[esafe]]  # [nch, 128, F]
            rows[~valid] = 0
            xg[:, ci:ci + nch, :] = rows.transpose(1, 0, 2)

            # layer-2 gather indices: fp8 row-pair entries; pads get
            # spread-out indices (identical pad indices hot-spot one bank)
            c_l2 = col2[esafe]
            assert np.all((c_l2 % 2)[valid] == h), "pos parity != src parity"
            spread = (np.arange(e.size, dtype=np.int64).reshape(e.shape) * 2654435761)
            i2 = np.where(valid, c_l2 // 2, spread % cfg.HALF2).astype(np.int64)
            assert i2.min() >= 0 and i2.max() < cfg.HALF2

            def wrap(ix):
                # ix [nch, 128] in chunk order -> flat order i -> tile[i%16, i//16]
                flat = ix.reshape(-1)
                tile = flat.reshape(-1, 16).T.astype(np.int16)  # [16, nch*8]
                return np.tile(tile, (8, 1))  # replicate to 128 partitions

            idx2_out.append(wrap(i2))

            # S[lane, ci+c, dstpos] = 8*w (padding lanes keep w=0)
            dl = np.where(valid, pos[dst_local[esafe]] % cfg.POS, 0)
            wv = np.where(valid, 8.0 * w[esafe], 0.0).astype(F8)
            lanes = np.broadcast_to(np.arange(128)[None, :], e.shape)
            cs = np.broadcast_to(np.arange(nch)[:, None], e.shape)
            s8[lanes.ravel(), (ci + cs).ravel(), dl.ravel()] = wv.ravel()
            ci += nch

    return {
        "xg": xg,
        "idx2": idx2_out,
        "s8": s8.reshape(128, -1),
    }


def host_preprocess(x, edge_index, edge_weight, gate_W, gate_U, gate_b, W0, cfg: Cfg):
    """Build all per-core device inputs. Returns (in_maps, meta)."""
    T, N, F = x.shape
    q = host_gru(gate_W, gate_U, gate_b, W0, T)  # [L,T,F,F] f32

    dst = np.asarray(edge_index[:, 0], dtype=np.int64)
    col = np.asarray(edge_index[:, 1], dtype=np.int64)
    w = np.asarray(edge_weight, dtype=np.float32)

    owner = dst // cfg.NPC

    # phase 1: pack every (core, t); collect pos maps
    pos_all = np.zeros((T, N), dtype=np.int64)
    packs = {}
    for t in range(T):
        for c in range(cfg.NCORES):
            m = owner[t] == c
            dl = dst[t][m] - c * cfg.NPC
            pos, perms = pack_core_t(dl, col[t][m], w[t][m], cfg)
            packs[(t, c)] = (dl, col[t][m], w[t][m], pos, perms)
            pos_all[t, c * cfg.NPC: (c + 1) * cfg.NPC] = pos

    # phase 2: per-core arrays.  The 8x edge-weight scaling inside S is
    # compensated by an 1/8 on every Q.
    qbf = (q / 8.0).astype(BF16)  # [L,T,F,F]
    xf8 = np.ascontiguousarray(x.astype(ml_dtypes.float8_e3m4))  # payload src

    in_maps = []
    for c in range(cfg.NCORES):
        xg_l, idx2_l, s8_l = [], [], []
        for t in range(T):
            dl, ct, wt, pos, perms = packs[(t, c)]

            def pos_global(carr, t=t):
                own = carr // cfg.NPC
                return own * cfg.POS_TOT + pos_all[t, carr]

            arrs = build_edge_arrays(dl, ct, wt, pos_global, pos, perms,
                                     xf8[t], cfg)
            xg_l.append(arrs["xg"])
            idx2_l.append(arrs["idx2"])
            s8_l.append(arrs["s8"])

        # idx tensor layout: [T, 128, IDXTOT] flat, per pass (A block then B)
        idx2 = np.zeros((T, 128, cfg.IDXTOT), dtype=np.int16)
        for t in range(T):
            for p in range(cfg.NPASS):
                o = cfg.idxoff(p)
                ch8 = cfg.chh(p) * 8
                idx2[t, :, o:o + ch8] = idx2_l[t][p * 2]
                idx2[t, :, o + ch8:o + 2 * ch8] = idx2_l[t][p * 2 + 1]

        in_maps.append({
            "xg": np.stack(xg_l),    # [T, 128, CHTOT, F] float8_e3m4
            "qmat": qbf,
            "idx2": idx2,
            "sfp8": np.stack(s8_l),  # [T, 128, CHTOT*POS] float8_e3m4
        })

    meta = {"pos_all": pos_all}
    return in_maps, meta


def host_assemble(results, pos_all, cfg: Cfg):
    """results: per-core dicts with 'out' [T, POS_TOT, F] f32 -> [T,N,F] f32."""
    T, N = pos_all.shape
    out = np.zeros((T, N, cfg.F), dtype=np.float32)
    for c, r in enumerate(results):
        dev = r["out"]  # [T, POS_TOT, F]
        for t in range(T):
            p = pos_all[t, c * cfg.NPC:(c + 1) * cfg.NPC]
            out[t, c * cfg.NPC:(c + 1) * cfg.NPC, :] = dev[t][p, :]
    return out


# ------------------------------------------------------------- bass program

def build_bass(cfg: Cfg):
    import concourse.mybir as mybir
    import concourse.tile as tile
    from concourse.bacc import Bacc

    fp32 = mybir.dt.float32
    bf16 = mybir.dt.bfloat16
    i16 = mybir.dt.int16
    f8e3 = mybir.dt.float8e3
    AF = mybir.ActivationFunctionType

    nc = Bacc(num_swdge_queues=4, dynamic_dma_scratch_size=32768)
    T, F = cfg.T, cfg.F

    xg = nc.declare_dram_parameter("xg", [T, 128, cfg.CHTOT, F], f8e3, isOutput=False)
    qmat = nc.declare_dram_parameter("qmat", [cfg.L, T, F, F], bf16, isOutput=False)
    idx2 = nc.declare_dram_parameter("idx2", [T, 128, cfg.IDXTOT], i16, isOutput=False)
    sfp8_in = nc.declare_dram_parameter(
        "sfp8", [T, 128, cfg.CHTOT * cfg.POS], f8e3, isOutput=False)
    out_d = nc.declare_dram_parameter("out", [T, cfg.POS_TOT, F], bf16, isOutput=True)

    # per-timestep scratch: layer-1 output slice + its AllGather.  One DRAM
    # tensor PER TIMESTEP so the tracker never sees AG(t)'s write and the
    # L2(t-1) gathers' reads as touching the same object (false serialization)
    kw = {"addr_space": "Shared"} if cfg.NCORES > 4 else {}
    t2own = [nc.dram_tensor(f"t2own{t}", [cfg.POS_TOT, F], f8e3)
             for t in range(T)]
    t2full = [nc.dram_tensor(f"t2full{t}", [cfg.NCORES * cfg.POS_TOT, F],
                             f8e3, **kw) for t in range(T)]
    groups = [list(range(cfg.NCORES))]

    with tile.TileContext(nc) as tc:
        with (
            tc.tile_pool(name="const", bufs=1) as constp,
            tc.tile_pool(name="meta", bufs=3) as metap,
            tc.tile_pool(name="msgf8", bufs=2) as msgf8p,
            tc.tile_pool(name="msgbf", bufs=3) as msgbfp,
            tc.tile_pool(name="sb", bufs=3) as sp,
            tc.tile_pool(name="stg", bufs=2) as stgp,
            tc.tile_pool(name="apsum", bufs=1, space="PSUM") as psp,
            tc.tile_pool(name="qpsum", bufs=4, space="PSUM") as hps,
        ):
            q_t = constp.tile([128, cfg.L * T * F], bf16)
            for l in range(cfg.L):
                for t in range(T):
                    o = (l * T + t) * F
                    nc.sync.dma_start(out=q_t[:, o:o + F], in_=qmat[l, t, :, :])

            qctr = [0]

            def qsl(l, t):
                o = (l * T + t) * F
                return q_t[:, o:o + F]

            def emit_pass(t, l, p):
                halfsz = cfg.HALF2
                chh = cfg.chh(p)  # chunks per half this pass
                cap = cfg.PCAPS[p]
                io = cfg.idxoff(p)
                so = cfg.soff(p)
                co = so // cfg.POS  # chunk offset of pass p

                if l == 0:
                    # layer 1: host pre-gathered payload, plain streaming
                    msg = msgf8p.tile([128, cfg.CHT, F], f8e3, tag="msg8")
                    nc.sync.dma_start(
                        out=msg[:, :2 * chh, :],
                        in_=xg[t, :, co:co + 2 * chh, :])
                else:
                    # layer 2: each 256 B entry is an (even,odd)-node fp8
                    # row pair; the chunk's class picks the feature half
                    msg = msgbfp.tile([128, cfg.CHT, 2 * F], f8e3, tag="msgb")
                    idxt = metap.tile([128, cfg.CHT * 8], i16, tag="idx")
                    nc.sync.dma_start(
                        out=idxt[:, :2 * chh * 8],
                        in_=idx2[t, :, io:io + 2 * chh * 8])
                    pairs = t2full[t][:, :].rearrange(
                        "(r two) f -> r (two f)", two=2)
                    for h in (0, 1):
                        nc.gpsimd.dma_gather(
                            msg[:, h * chh:(h + 1) * chh, :],
                            pairs,
                            idxt[:, h * chh * 8:(h + 1) * chh * 8],
                            num_idxs=chh * 128,
                            num_idxs_reg=chh * 128,
                            elem_size=2 * F,
                            single_packet=False,
                            queue_num=qctr[0] % 4,
                        )
                        qctr[0] += 1

                # one-hot S with 8*w folded in, host-built fp8 (e3m4)
                S = sp.tile([128, cfg.CHT, cfg.POS], f8e3, tag="S")
                nc.sync.dma_start(
                    out=S[:, :2 * chh, :],
                    in_=sfp8_in[t, :, so:so + 2 * chh * cfg.POS].rearrange(
                        "p (c j) -> p c j", j=cfg.POS),
                )

                # aggregate per table half into separate PSUM tiles so the
                # A-half matmul burst overlaps the B gather's tail; the Q
                # matmul below sums the halves via PE accumulation.
                bsbh = []
                for h in (0, 1):
                    agg = psp.tile([128, cfg.PASS_W], fp32, tag=f"agg{h}")
                    for si in range(cfg.SLOTS):
                        for k in range(cap):
                            ci = h * chh + si * cap + k
                            lhsT = (msg[:, ci, :] if l == 0 else
                                    msg[:, ci, h * F:(h + 1) * F])
                            nc.tensor.matmul(
                                out=agg[:, si * cfg.POS:(si + 1) * cfg.POS],
                                lhsT=lhsT,
                                rhs=S[:, ci, :],
                                start=(k == 0),
                                stop=(k == cap - 1),
                            )
                    bsb = sp.tile([128, cfg.PASS_W], bf16, tag=f"bsb{h}")
                    nc.scalar.activation(out=bsb[:], in_=agg[:], func=AF.Copy)
                    bsbh.append(bsb)
                emit_tail(t, l, p, bsbh)

            def emit_tail(t, l, p, bsbh):
                # node-major (B @ Q) in 128-row blocks, rrelu, store; the two
                # aggregation halves sum via a 2-matmul accumulation group
                for j0 in range(0, cfg.PASS_W // 128, 4):
                    qp = hps.tile([128, 4 * F], fp32, tag="qp")
                    for jj in range(4):
                        j = j0 + jj
                        for h, bsb in enumerate(bsbh):
                            nc.tensor.matmul(
                                out=qp[:, jj * F:(jj + 1) * F],
                                lhsT=bsb[:, j * 128:(j + 1) * 128],
                                rhs=qsl(l, t),
                                start=(h == 0), stop=(h == len(bsbh) - 1),
                            )
                    lk = stgp.tile([128, 4 * F], fp32, tag="lk")
                    nc.scalar.activation(out=lk[:], in_=qp[:], func=AF.Copy, scale=NEG_SLOPE)
                    r0 = p * cfg.PASS_W + j0 * 128
                    # store via the Scalar engine so the Sync queue stays
                    # pure-loads (no head-of-line blocking of S/idx loads
                    # behind compute-dependent stores)
                    if l == 0:
                        st = stgp.tile([128, 4 * F], f8e3, tag="st1")
                        dest = t2own[t][r0:r0 + 512, :]
                    else:
                        st = stgp.tile([128, 4 * F], bf16, tag="st2")
                        dest = out_d[t, r0:r0 + 512, :]
                    nc.vector.tensor_tensor(
                        out=st[:], in0=qp[:], in1=lk[:], op=mybir.AluOpType.max,
                    )
                    nc.scalar.dma_start(
                        out=dest.rearrange("(j p) f -> p j f", p=128),
                        in_=st[:].rearrange("p (j f) -> p j f", j=4),
                    )

            # Emission order: L1(t) streams while L2(t-1) gathers; the
            # AllGather(t) trigger (a GpSimd-queue instruction that waits on
            # L1(t)'s stores) is sandwiched between early L2(t-1) passes so
            # the gather queue never head-of-line-stalls on it, while the
            # remaining L2(t-1) passes cover the collective's latency before
            # L2(t) needs t2full[t].
            def emit_ag(t):
                nc.gpsimd.collective_compute(
                    "AllGather", mybir.AluOpType.bypass,
                    replica_groups=groups,
                    ins=[t2own[t][:, :]], outs=[t2full[t][:, :]],
                )

            for t in range(T):
                for p in range(cfg.NPASS):
                    emit_pass(t, 0, p)
                if t == 0:
                    emit_ag(0)
                else:
                    emit_pass(t - 1, 1, 0)
                    emit_ag(t)
                    for p in range(1, cfg.NPASS):
                        emit_pass(t - 1, 1, p)
            for p in range(cfg.NPASS):
                emit_pass(T - 1, 1, p)
    nc.finalize()
    return nc


# ------------------------------------------------------------------- driver

TRACE = False
LAST_RESULT = None


def kernel(x, edge_index, edge_weight, gate_W, gate_U, gate_b, W0):
    global LAST_RESULT
    from concourse.bass_utils import run_bass_kernel_spmd

    cfg = CFG
    x = np.asarray(x)
    in_maps, meta = host_preprocess(
        x, np.asarray(edge_index), np.asarray(edge_weight),
        np.asarray(gate_W), np.asarray(gate_U), np.asarray(gate_b),
        np.asarray(W0), cfg,
    )
    nc = build_bass(cfg)
    res = run_bass_kernel_spmd(nc, in_maps, list(range(cfg.NCORES)), trace=TRACE)
    LAST_RESULT = res
    return host_assemble(res.results, meta["pos_all"], cfg).astype(np.float32)



# revision 45
# speedup vs baseline: 1.2135x; 1.0128x over previous
"""EvolveGCN Trainium2 kernel (8-core SPMD), aggregate-first design.

Math: out_l = rrelu(segment_sum(w_e * x_l[src_e]) @ Q_l) -- the GCN weight
multiply commutes with the (linear) edge aggregation, so we aggregate raw
feature rows first and apply Q to the [N, F] aggregate afterwards.

Strategy:
  - Nodes/edges sharded by destination across 8 cores (6250 nodes each).
  - GRU weight evolution (128x128 mats) computed on the host.
  - Layer 1 payload rows are pre-gathered on the HOST into exact
    (lane, chunk) order (the gather indices are pure input data), so the
    device just streams them with large contiguous DMAs -- zero SWDGE
    descriptor generation and no small-descriptor HBM penalty for layer 1.
  - Layer 2 needs the layer-1 output of all cores: one bf16 AllGather per
    timestep, then true dma_gather (device-dependent data).
  - Aggregation uses PE matmuls: gathered 128-edge chunks (lhsT) times
    one-hot selection matrices S (rhs, edge-weight folded in) accumulate
    into per-slot PSUM columns.  S is built on the Vector engine from
    per-edge (slot-position, weight) metadata via iota compare.
  - The @Q product then maps the [F, pos] aggregate back to node-major
    [pos, F] PSUM tiles, which store as contiguous rows.
  - Host bin-packs destination nodes into fixed-capacity slots (64 node
    positions, 4+4 chunks of 128 edges for the two gather-table halves --
    int16 gather indices force a half split) so the device program is
    fully static; padding edges have w=0 and spread-out gather indices.
  - Emission interleaves timesteps (L1(t) ... AG(t) ... L2(t-1)) so the
    gather queues stay saturated while collectives run.
"""

import sys

for _p in ("/opt/trn_rl_repo", "/opt/pypackages"):
    if _p not in sys.path:
        sys.path.append(_p)

from dataclasses import dataclass

import numpy as np
import ml_dtypes

BF16 = ml_dtypes.bfloat16
NEG_SLOPE = (1.0 / 8.0 + 1.0 / 3.0) / 2.0


@dataclass(frozen=True)
class Cfg:
    T: int = 6
    N: int = 50000
    F: int = 128
    L: int = 2
    NCORES: int = 8
    POS: int = 64         # node positions per slot
    # chunks (of 128 edges) per slot per table half, by pass: the last pass
    # is half-capacity, trimming total edge slots 114688 -> 106496 (~7%)
    PCAPS: tuple = (4, 4, 4, 4, 4, 3, 3)
    SLOTS: int = 16       # slots per psum pass
    NPASS: int = 7

    @property
    def NPC(self):  # nodes per core (dst sharding)
        return self.N // self.NCORES

    @property
    def NSLOT(self):
        return self.SLOTS * self.NPASS

    @property
    def POS_TOT(self):  # padded positions per core
        return self.NSLOT * self.POS

    @property
    def PASS_W(self):  # psum width per pass
        return self.SLOTS * self.POS

    @property
    def CHT(self):  # max chunks per pass (A block + B block)
        return 2 * max(self.PCAPS) * self.SLOTS

    @property
    def CHTOT(self):  # total chunks over all passes
        return 2 * sum(self.PCAPS) * self.SLOTS

    @property
    def IDXTOT(self):  # total idx columns over all passes (A+B)
        return self.CHTOT * 8

    def chh(self, p):  # chunks per half in pass p
        return self.PCAPS[p] * self.SLOTS

    def idxoff(self, p):  # idx column offset of pass p
        return 2 * 8 * self.SLOTS * sum(self.PCAPS[:p])

    def soff(self, p):  # S element offset of pass p
        return 2 * self.SLOTS * self.POS * sum(self.PCAPS[:p])

    @property
    def HALF1(self):  # layer-1 gather table half size
        return self.N // 2

    @property
    def HALF2(self):  # layer-2 table half size (= cores 0..NC/2-1)
        return self.NCORES * self.POS_TOT // 2


CFG = Cfg()


# ----------------------------------------------------------------- host math

def host_gru(gate_W, gate_U, gate_b, W0, T):
    """Evolve the GCN weight through the GRU on the host.

    Returns Q[l, t] float32 [L, T, F, F]."""
    L = gate_W.shape[0]
    F = W0.shape[-1]
    out = np.zeros((L, T, F, F), dtype=np.float32)

    def sigmoid(v):
        return 1.0 / (1.0 + np.exp(-v))

    for l in range(L):
        Q = W0[l].astype(np.float32)
        gW, gU, gb = (np.asarray(a[l], dtype=np.float32) for a in (gate_W, gate_U, gate_b))
        for t in range(T):
            z = sigmoid(gW[0] @ Q + gU[0] @ Q + gb[0])
            r = sigmoid(gW[1] @ Q + gU[1] @ Q + gb[1])
            h = np.tanh(gW[2] @ Q + gU[2] @ (r * Q) + gb[2])
            Q = (1.0 - z) * Q + z * h
            out[l, t] = Q
    return out


def pack_core_t(dst_local, col, w, cfg: Cfg):
    """Bin-pack one core's edges at one timestep into the static slot layout.

    Returns (pos, perms) where
      pos:   [NPC] position assigned to each local node
      perms: per (pass, half) -> int array of edge ids (-1 for padding) laid
             out as [CH*128] in chunk order (slot-major).
    """
    # Edge class (A/B chunk block) = parity of the SOURCE node id.  The
    # layer-2 gather table holds fp8 row PAIRS (even-parity node at row 2k,
    # odd at 2k+1), so a 256 B gather entry serves either class and the
    # chunk's class picks which feature half of the pair to feed the PE --
    # a compile-time constant, identical on every core.
    half = (col % 2).astype(np.int8)

    # per-node degree split by class
    degA = np.bincount(dst_local[half == 0], minlength=cfg.NPC)
    degB = np.bincount(dst_local[half == 1], minlength=cfg.NPC)

    caps = np.repeat(np.asarray(cfg.PCAPS, dtype=np.int64), cfg.SLOTS) * 128
    remA = caps.copy()
    remB = caps.copy()
    # positions of each parity per slot (node parity must equal pos parity)
    remNe = np.full(cfg.NSLOT, cfg.POS // 2, dtype=np.int64)
    remNo = np.full(cfg.NSLOT, cfg.POS // 2, dtype=np.int64)

    order = np.argsort(-(degA + degB), kind="stable")
    slot_of = np.empty(cfg.NPC, dtype=np.int64)
    pos_in_slot = np.empty(cfg.NPC, dtype=np.int64)
    for n in order:
        da, db = degA[n], degB[n]
        remN = remNe if n % 2 == 0 else remNo
        ok = np.flatnonzero((remA >= da) & (remB >= db) & (remN > 0))
        if ok.size == 0:
            raise RuntimeError("bin packing failed; increase capacity")
        s = ok[0]
        slot_of[n] = s
        if n % 2 == 0:
            pos_in_slot[n] = 2 * (cfg.POS // 2 - remNe[s])
            remNe[s] -= 1
        else:
            pos_in_slot[n] = 2 * (cfg.POS // 2 - remNo[s]) + 1
            remNo[s] -= 1
        remA[s] -= da
        remB[s] -= db
    pos = slot_of * cfg.POS + pos_in_slot

    # assign edges to chunk lanes: order edges by (slot, half)
    eslot = slot_of[dst_local]
    key = eslot * 2 + half
    eorder = np.argsort(key, kind="stable")
    counts = np.bincount(key, minlength=cfg.NSLOT * 2)
    starts = np.concatenate(([0], np.cumsum(counts)))

    perms = []
    for p in range(cfg.NPASS):
        for h in (0, 1):
            ch = cfg.PCAPS[p]
            blk = np.full(cfg.SLOTS * ch * 128, -1, dtype=np.int64)
            for si in range(cfg.SLOTS):
                s = p * cfg.SLOTS + si
                k = s * 2 + h
                seg = eorder[starts[k]:starts[k + 1]]
                assert seg.size <= ch * 128
                blk[si * ch * 128: si * ch * 128 + seg.size] = seg
            perms.append(blk)
    return pos, perms


def build_edge_arrays(dst_local, col, w, pos_global_fn, pos, perms, xf8_t, cfg: Cfg):
    """Build per-(core,t) device arrays from a packing.

    Returns dict with
      xg:   [128, CHTOT, F] float8_e3m4 layer-1 edge payloads, pre-gathered
            on the host into exact (lane, chunk) order -- the device streams
            these with plain contiguous DMA (zero gather descriptors for
            layer 1)
      idx2: list of [128, nch*8] int16 (order pass,half) layer-2 gather idx
      s8:   one-hot selection matrices [128, CHTOT, POS] float8_e3m4 holding
            8*w at each edge's destination position (the 8x keeps small
            weights out of the fp8 denormal range; the host folds 1/8 into
            Q).
    """
    col2 = pos_global_fn(col)
    F8 = ml_dtypes.float8_e3m4
    s8 = np.zeros((128, cfg.CHTOT, cfg.POS), dtype=F8)
    xg = np.zeros((128, cfg.CHTOT, cfg.F), dtype=F8)
    idx2_out = []

    ci = 0  # global chunk index in [0, CHTOT)
    for p in range(cfg.NPASS):
        for bi, h in enumerate((0, 1)):
            ch = cfg.PCAPS[p]
            blk = perms[p * 2 + bi]
            nch = cfg.SLOTS * ch
            e = blk.reshape(nch, 128)
            valid = e >= 0
            esafe = np.where(valid, e, 0)

            # layer-1 payload rows, pre-gathered host-side (pads stay zero)
            rows = xf8_t[col[esafe]]  # [nch, 128, F]
            rows[~valid] = 0
            xg[:, ci:ci + nch, :] = rows.transpose(1, 0, 2)

            # layer-2 gather indices: fp8 row-pair entries; pads get
            # spread-out indices (identical pad indices hot-spot one bank)
            c_l2 = col2[esafe]
            assert np.all((c_l2 % 2)[valid] == h), "pos parity != src parity"
            spread = (np.arange(e.size, dtype=np.int64).reshape(e.shape) * 2654435761)
            i2 = np.where(valid, c_l2 // 2, spread % cfg.HALF2).astype(np.int64)
            assert i2.min() >= 0 and i2.max() < cfg.HALF2

            def wrap(ix):
                # ix [nch, 128] in chunk order -> flat order i -> tile[i%16, i//16]
                flat = ix.reshape(-1)
                tile = flat.reshape(-1, 16).T.astype(np.int16)  # [16, nch*8]
                return np.tile(tile, (8, 1))  # replicate to 128 partitions

            idx2_out.append(wrap(i2))

            # S[lane, ci+c, dstpos] = 8*w (padding lanes keep w=0)
            dl = np.where(valid, pos[dst_local[esafe]] % cfg.POS, 0)
            wv = np.where(valid, 8.0 * w[esafe], 0.0).astype(F8)
            lanes = np.broadcast_to(np.arange(128)[None, :], e.shape)
            cs = np.broadcast_to(np.arange(nch)[:, None], e.shape)
            s8[lanes.ravel(), (ci + cs).ravel(), dl.ravel()] = wv.ravel()
            ci += nch

    return {
        "xg": xg,
        "idx2": idx2_out,
        "s8": s8.reshape(128, -1),
    }


def host_preprocess(x, edge_index, edge_weight, gate_W, gate_U, gate_b, W0, cfg: Cfg):
    """Build all per-core device inputs. Returns (in_maps, meta)."""
    T, N, F = x.shape
    q = host_gru(gate_W, gate_U, gate_b, W0, T)  # [L,T,F,F] f32

    dst = np.asarray(edge_index[:, 0], dtype=np.int64)
    col = np.asarray(edge_index[:, 1], dtype=np.int64)
    w = np.asarray(edge_weight, dtype=np.float32)

    owner = dst // cfg.NPC

    # phase 1: pack every (core, t); collect pos maps
    pos_all = np.zeros((T, N), dtype=np.int64)
    packs = {}
    for t in range(T):
        for c in range(cfg.NCORES):
            m = owner[t] == c
            dl = dst[t][m] - c * cfg.NPC
            pos, perms = pack_core_t(dl, col[t][m], w[t][m], cfg)
            packs[(t, c)] = (dl, col[t][m], w[t][m], pos, perms)
            pos_all[t, c * cfg.NPC: (c + 1) * cfg.NPC] = pos

    # phase 2: per-core arrays.  The 8x edge-weight scaling inside S is
    # compensated by an 1/8 on every Q.
    qbf = (q / 8.0).astype(BF16)  # [L,T,F,F]
    xf8 = np.ascontiguousarray(x.astype(ml_dtypes.float8_e3m4))  # payload src

    in_maps = []
    for c in range(cfg.NCORES):
        xg_l, idx2_l, s8_l = [], [], []
        for t in range(T):
            dl, ct, wt, pos, perms = packs[(t, c)]

            def pos_global(carr, t=t):
                own = carr // cfg.NPC
                return own * cfg.POS_TOT + pos_all[t, carr]

            arrs = build_edge_arrays(dl, ct, wt, pos_global, pos, perms,
                                     xf8[t], cfg)
            xg_l.append(arrs["xg"])
            idx2_l.append(arrs["idx2"])
            s8_l.append(arrs["s8"])

        # idx tensor layout: [T, 128, IDXTOT] flat, per pass (A block then B)
        idx2 = np.zeros((T, 128, cfg.IDXTOT), dtype=np.int16)
        for t in range(T):
            for p in range(cfg.NPASS):
                o = cfg.idxoff(p)
                ch8 = cfg.chh(p) * 8
                idx2[t, :, o:o + ch8] = idx2_l[t][p * 2]
                idx2[t, :, o + ch8:o + 2 * ch8] = idx2_l[t][p * 2 + 1]

        in_maps.append({
            "xg": np.stack(xg_l),    # [T, 128, CHTOT, F] float8_e3m4
            "qmat": qbf,
            "idx2": idx2,
            "sfp8": np.stack(s8_l),  # [T, 128, CHTOT*POS] float8_e3m4
        })

    meta = {"pos_all": pos_all}
    return in_maps, meta


def host_assemble(results, pos_all, cfg: Cfg):
    """results: per-core dicts with 'out' [T, POS_TOT, F] f32 -> [T,N,F] f32."""
    T, N = pos_all.shape
    out = np.zeros((T, N, cfg.F), dtype=np.float32)
    for c, r in enumerate(results):
        dev = r["out"]  # [T, POS_TOT, F]
        for t in range(T):
            p = pos_all[t, c * cfg.NPC:(c + 1) * cfg.NPC]
            out[t, c * cfg.NPC:(c + 1) * cfg.NPC, :] = dev[t][p, :]
    return out


# ------------------------------------------------------------- bass program

def build_bass(cfg: Cfg):
    import concourse.mybir as mybir
    import concourse.tile as tile
    from concourse.bacc import Bacc

    fp32 = mybir.dt.float32
    bf16 = mybir.dt.bfloat16
    i16 = mybir.dt.int16
    f8e3 = mybir.dt.float8e3
    AF = mybir.ActivationFunctionType

    nc = Bacc(num_swdge_queues=4, dynamic_dma_scratch_size=32768)
    T, F = cfg.T, cfg.F

    xg = nc.declare_dram_parameter("xg", [T, 128, cfg.CHTOT, F], f8e3, isOutput=False)
    qmat = nc.declare_dram_parameter("qmat", [cfg.L, T, F, F], bf16, isOutput=False)
    idx2 = nc.declare_dram_parameter("idx2", [T, 128, cfg.IDXTOT], i16, isOutput=False)
    sfp8_in = nc.declare_dram_parameter(
        "sfp8", [T, 128, cfg.CHTOT * cfg.POS], f8e3, isOutput=False)
    out_d = nc.declare_dram_parameter("out", [T, cfg.POS_TOT, F], bf16, isOutput=True)

    # per-timestep scratch: layer-1 output slice + its AllGather.  One DRAM
    # tensor PER TIMESTEP so the tracker never sees AG(t)'s write and the
    # L2(t-1) gathers' reads as touching the same object (false serialization)
    kw = {"addr_space": "Shared"} if cfg.NCORES > 4 else {}
    t2own = [nc.dram_tensor(f"t2own{t}", [cfg.POS_TOT, F], f8e3)
             for t in range(T)]
    t2full = [nc.dram_tensor(f"t2full{t}", [cfg.NCORES * cfg.POS_TOT, F],
                             f8e3, **kw) for t in range(T)]
    groups = [list(range(cfg.NCORES))]

    with tile.TileContext(nc) as tc:
        with (
            tc.tile_pool(name="const", bufs=1) as constp,
            tc.tile_pool(name="meta", bufs=3) as metap,
            tc.tile_pool(name="msgf8", bufs=2) as msgf8p,
            tc.tile_pool(name="msgbf", bufs=3) as msgbfp,
            tc.tile_pool(name="sb", bufs=3) as sp,
            tc.tile_pool(name="stg", bufs=2) as stgp,
            tc.tile_pool(name="apsum", bufs=1, space="PSUM") as psp,
            tc.tile_pool(name="qpsum", bufs=4, space="PSUM") as hps,
        ):
            q_t = constp.tile([128, cfg.L * T * F], bf16)
            for l in range(cfg.L):
                for t in range(T):
                    o = (l * T + t) * F
                    nc.sync.dma_start(out=q_t[:, o:o + F], in_=qmat[l, t, :, :])

            qctr = [0]

            def qsl(l, t):
                o = (l * T + t) * F
                return q_t[:, o:o + F]

            def emit_pass(t, l, p):
                halfsz = cfg.HALF2
                chh = cfg.chh(p)  # chunks per half this pass
                cap = cfg.PCAPS[p]
                io = cfg.idxoff(p)
                so = cfg.soff(p)
                co = so // cfg.POS  # chunk offset of pass p

                if l == 0:
                    # layer 1: host pre-gathered payload, plain streaming
                    msg = msgf8p.tile([128, cfg.CHT, F], f8e3, tag="msg8")
                    nc.sync.dma_start(
                        out=msg[:, :2 * chh, :],
                        in_=xg[t, :, co:co + 2 * chh, :])
                else:
                    # layer 2: each 256 B entry is an (even,odd)-node fp8
                    # row pair; the chunk's class picks the feature half
                    msg = msgbfp.tile([128, cfg.CHT, 2 * F], f8e3, tag="msgb")
                    idxt = metap.tile([128, cfg.CHT * 8], i16, tag="idx")
                    nc.sync.dma_start(
                        out=idxt[:, :2 * chh * 8],
                        in_=idx2[t, :, io:io + 2 * chh * 8])
                    pairs = t2full[t][:, :].rearrange(
                        "(r two) f -> r (two f)", two=2)
                    for h in (0, 1):
                        nc.gpsimd.dma_gather(
                            msg[:, h * chh:(h + 1) * chh, :],
                            pairs,
                            idxt[:, h * chh * 8:(h + 1) * chh * 8],
                            num_idxs=chh * 128,
                            num_idxs_reg=chh * 128,
                            elem_size=2 * F,
                            single_packet=False,
                            queue_num=qctr[0] % 4,
                        )
                        qctr[0] += 1

                # one-hot S with 8*w folded in, host-built fp8 (e3m4)
                S = sp.tile([128, cfg.CHT, cfg.POS], f8e3, tag="S")
                nc.sync.dma_start(
                    out=S[:, :2 * chh, :],
                    in_=sfp8_in[t, :, so:so + 2 * chh * cfg.POS].rearrange(
                        "p (c j) -> p c j", j=cfg.POS),
                )

                # aggregate per table half into separate PSUM tiles so the
                # A-half matmul burst overlaps the B gather's tail; the Q
                # matmul below sums the halves via PE accumulation.
                bsbh = []
                for h in (0, 1):
                    agg = psp.tile([128, cfg.PASS_W], fp32, tag=f"agg{h}")
                    for si in range(cfg.SLOTS):
                        for k in range(cap):
                            ci = h * chh + si * cap + k
                            lhsT = (msg[:, ci, :] if l == 0 else
                                    msg[:, ci, h * F:(h + 1) * F])
                            nc.tensor.matmul(
                                out=agg[:, si * cfg.POS:(si + 1) * cfg.POS],
                                lhsT=lhsT,
                                rhs=S[:, ci, :],
                                start=(k == 0),
                                stop=(k == cap - 1),
                            )
                    bsb = sp.tile([128, cfg.PASS_W], bf16, tag=f"bsb{h}")
                    nc.scalar.activation(out=bsb[:], in_=agg[:], func=AF.Copy)
                    bsbh.append(bsb)
                emit_tail(t, l, p, bsbh)

            def emit_tail(t, l, p, bsbh):
                # node-major (B @ Q) in 128-row blocks, rrelu, store; the two
                # aggregation halves sum via a 2-matmul accumulation group
                for j0 in range(0, cfg.PASS_W // 128, 4):
                    qp = hps.tile([128, 4 * F], fp32, tag="qp")
                    for jj in range(4):
                        j = j0 + jj
                        for h, bsb in enumerate(bsbh):
                            nc.tensor.matmul(
                                out=qp[:, jj * F:(jj + 1) * F],
                                lhsT=bsb[:, j * 128:(j + 1) * 128],
                                rhs=qsl(l, t),
                                start=(h == 0), stop=(h == len(bsbh) - 1),
                            )
                    lk = stgp.tile([128, 4 * F], fp32, tag="lk")
                    nc.scalar.activation(out=lk[:], in_=qp[:], func=AF.Copy, scale=NEG_SLOPE)
                    r0 = p * cfg.PASS_W + j0 * 128
                    # store via the Scalar engine so the Sync queue stays
                    # pure-loads (no head-of-line blocking of S/idx loads
                    # behind compute-dependent stores)
                    if l == 0:
                        st = stgp.tile([128, 4 * F], f8e3, tag="st1")
                        dest = t2own[t][r0:r0 + 512, :]
                    else:
                        st = stgp.tile([128, 4 * F], bf16, tag="st2")
                        dest = out_d[t, r0:r0 + 512, :]
                    nc.vector.tensor_tensor(
                        out=st[:], in0=qp[:], in1=lk[:], op=mybir.AluOpType.max,
                    )
                    nc.scalar.dma_start(
                        out=dest.rearrange("(j p) f -> p j f", p=128),
                        in_=st[:].rearrange("p (j f) -> p j f", j=4),
                    )

            # Emission order: L1(t) streams while L2(t-1) gathers; the
            # AllGather(t) trigger (a GpSimd-queue instruction that waits on
            # L1(t)'s stores) is sandwiched between early L2(t-1) passes so
            # the gather queue never head-of-line-stalls on it, while the
            # remaining L2(t-1) passes cover the collective's latency before
            # L2(t) needs t2full[t].
            def emit_ag(t):
                nc.gpsimd.collective_compute(
                    "AllGather", mybir.AluOpType.bypass,
                    replica_groups=groups,
                    ins=[t2own[t][:, :]], outs=[t2full[t][:, :]],
                )

            for t in range(T):
                for p in range(cfg.NPASS):
                    emit_pass(t, 0, p)
                if t == 0:
                    emit_ag(0)
                else:
                    emit_pass(t - 1, 1, 0)
                    emit_pass(t - 1, 1, 1)
                    emit_pass(t - 1, 1, 2)
                    emit_ag(t)
                    for p in range(3, cfg.NPASS):
                        emit_pass(t - 1, 1, p)
            for p in range(cfg.NPASS):
                emit_pass(T - 1, 1, p)
    nc.finalize()
    return nc


# ------------------------------------------------------------------- driver

TRACE = False
LAST_RESULT = None


def kernel(x, edge_index, edge_weight, gate_W, gate_U, gate_b, W0):
    global LAST_RESULT
    from concourse.bass_utils import run_bass_kernel_spmd

    cfg = CFG
    x = np.asarray(x)
    in_maps, meta = host_preprocess(
        x, np.asarray(edge_index), np.asarray(edge_weight),
        np.asarray(gate_W), np.asarray(gate_U), np.asarray(gate_b),
        np.asarray(W0), cfg,
    )
    nc = build_bass(cfg)
    res = run_bass_kernel_spmd(nc, in_maps, list(range(cfg.NCORES)), trace=TRACE)
    LAST_RESULT = res
    return host_assemble(res.results, meta["pos_all"], cfg).astype(np.float32)

